# revision 25
# baseline (speedup 1.0000x reference)
"""AttentionPairBias kernel for Trainium2, 8-core SPMD.

Math (per batch=1):
  pn        = LayerNorm(pairwise) * gamma + beta                  [N, N, 128]
  attn_bias = einsum('ijp,ph->hij', pn, W_bias)                   [16, N, N]
  q,k,v     = single @ Wq/Wk/Wv  (split into 16 heads of 64)
  scores    = q k^T / sqrt(64) + attn_bias ; attn = softmax_j
  o         = attn @ v ; out = (o * sigmoid(single@Wg + bg)) @ Wo [N, 1024]

Sharding: rows of i (queries) across 8 cores; k/v compute replicated.

v2 design:
  - pairwise is pre-cast to bf16 on the host; the per-core slice
    [128, N, 128] streams in via HWDGE xbar DMA-transpose straight into
    SBUF as oct[p, j, i] tiles (no PE transposes, no PSUM evacuation).
  - LN mean-term is folded into the weights: W2 = gamma*W_bias - colsum/128,
    so bias = rinv * (x @ W2).  The projection matmul streams [W2 | ones]
    (17 cols) per j; a second 1-col matmul vs ones over octsq gives sumsq.
    Both land in one PSUM tile [128, JB, 18]: cols 0-15 bias, 16 sums,
    17 sumsq.  beta@W_bias is constant over j and cancels in softmax.
  - Per-quarter stats produce rinv (bf16); one 2x-mode DVE mul applies it.
  - Phase A (q/k/v/gates) is emitted interleaved with Phase B chunks so its
    matmuls fill the Tensor engine while B is DMA-paced.
  - Phase C softmax: bias is accumulated into the qk PSUM via an
    identity-stationary matmul; ACT exp+accum reads PSUM directly.
"""

import numpy as np
import ml_dtypes

import concourse.bacc as bacc
import concourse.bass as bass
import concourse.tile as tile
import concourse.mybir as mybir
from concourse.bass_utils import run_bass_kernel_spmd
from concourse.masks import make_identity

N, DIM, HEADS, DHEAD, DPAIR = 1024, 1024, 16, 64, 128
NCORES = 8
IBLK = N // NCORES  # 128
EPS = 1e-5

JB = 16             # j-columns per Phase-B chunk
NCHUNK = N // JB
OCT_BUFS = 8

F32 = mybir.dt.float32
BF16 = mybir.dt.bfloat16
AX = mybir.AxisListType
AF = mybir.ActivationFunctionType
OP = mybir.AluOpType
BFNP = ml_dtypes.bfloat16


def _insert_bcast(ap, count, pos):
    """Insert a zero-stride broadcast dim of length `count` at free-dim
    position `pos` (0 = right after the partition dim)."""
    l = list(ap.ap)
    l.insert(1 + pos, [0, count])
    return bass.AP(tensor=ap.tensor, offset=ap.offset, ap=l)


def _row_tiles(dram_t, ncols):
    """AP viewing a [R, ncols] DRAM matrix as [128, R//128, ncols]:
    partition p, free (ct, c) -> dram[ct*128 + p, c]."""
    base = dram_t[:, :]
    nrows = base.ap[0][1]
    return bass.AP(tensor=base.tensor, offset=base.offset,
                   ap=[[ncols, 128], [128 * ncols, nrows // 128], [1, ncols]])


def _swap_free(ap):
    """Swap the two free dims of a 3D AP (iteration-order change)."""
    l = list(ap.ap)
    assert len(l) == 3
    return bass.AP(tensor=ap.tensor, offset=ap.offset, ap=[l[0], l[2], l[1]])


def build_program(reps=1, tiny_out=False):
    nc = bacc.Bacc("TRN2", target_bir_lowering=False, debug=False)

    pairb = nc.dram_tensor("pairb", [DPAIR, N, IBLK], BF16, kind="ExternalInput")
    sT = nc.dram_tensor("sT", [DIM, N], BF16, kind="ExternalInput")
    sTi = nc.dram_tensor("sTi", [DIM, IBLK], BF16, kind="ExternalInput")
    wq = nc.dram_tensor("wq", [DIM, DIM], BF16, kind="ExternalInput")
    wk = nc.dram_tensor("wk", [DIM, DIM], BF16, kind="ExternalInput")
    wv = nc.dram_tensor("wv", [DIM, DIM], BF16, kind="ExternalInput")
    wg = nc.dram_tensor("wg", [DIM, DIM], BF16, kind="ExternalInput")
    wo = nc.dram_tensor("wo", [DIM, DIM], BF16, kind="ExternalInput")
    weff = nc.dram_tensor("weff", [DPAIR, HEADS + 1], BF16, kind="ExternalInput")
    bgt = nc.dram_tensor("bgt", [128, 8], F32, kind="ExternalInput")
    out_cols = 8 if tiny_out else DIM
    out = nc.dram_tensor("out", [IBLK, out_cols], F32, kind="ExternalOutput")

    CT = DIM // 128  # 8 contraction tiles

    with tile.TileContext(nc) as tc:
        with tc.tile_pool(name="consts", bufs=1) as consts, \
             tc.tile_pool(name="persist", bufs=1) as pers:
            ident = consts.tile([128, 128], BF16, tag="ident", name="ident")
            make_identity(nc, ident)
            ones1 = consts.tile([128, 1], BF16, tag="ones1", name="ones1")
            nc.vector.memset(ones1, 1.0)
            weff_sb = consts.tile([DPAIR, HEADS + 1], BF16, tag="weff", name="weff")
            nc.sync.dma_start(out=weff_sb, in_=weff[:, :])
            bgt_sb = consts.tile([128, 8], F32, tag="bgt", name="bgt")
            nc.sync.dma_start(out=bgt_sb, in_=bgt[:, :])

            for _rep in range(reps):
                # persistent tensors
                kT = [pers.tile([128, N], BF16, tag=f"kT{t}", name=f"kT{t}") for t in range(8)]
                vsb = [pers.tile([128, DIM], BF16, tag=f"v{t}", name=f"v{t}") for t in range(8)]
                qT = [pers.tile([128, IBLK], BF16, tag=f"qT{t}", name=f"qT{t}") for t in range(8)]
                gT = [pers.tile([128, IBLK], F32, tag=f"gT{t}", name=f"gT{t}") for t in range(8)]
                bias_h = pers.tile([128, HEADS, N], BF16, tag="biasH", name="biasH")
                stats = pers.tile([128, 2, N], F32, tag="stats", name="stats")

                # ------- Phases B (pairwise bias) + A (projections), fused ----
                with tc.tile_pool(name="pb", bufs=1) as pb, \
                     tc.tile_pool(name="psB", bufs=2, space="PSUM") as psB, \
                     tc.tile_pool(name="pa", bufs=1) as pa, \
                     tc.tile_pool(name="psA", bufs=2, space="PSUM") as psA:
                    eps4 = pb.tile([128, 1], F32, tag="eps4", name="eps4")
                    nc.vector.memset(eps4, EPS)

                    # Phase A inputs (DMAs deferred into the chunk loop so
                    # the pairwise stream gets the DMA engines first)
                    s_sb = []
                    si_sb = []

                    def emit_s_dmas():
                        s4 = pa.tile([128, CT, N], BF16, tag="s4", name="s4")
                        nc.scalar.dma_start(out=s4, in_=_row_tiles(sT, N))
                        s_sb.extend(s4[:, ct, :] for ct in range(CT))
                        si4 = pa.tile([128, CT, IBLK], BF16, tag="si4",
                                      name="si4")
                        nc.scalar.dma_start(out=si4, in_=_row_tiles(sTi, IBLK))
                        si_sb.extend(si4[:, ct, :] for ct in range(CT))
                    wsb = {}

                    def emit_weight_dmas(pairs):
                        # rotating 2-buf tag: k/v fill the two bufs, q/g reuse
                        # them once the kT/v matmul units have drained
                        for nm, dram in pairs:
                            wt = pa.tile([128, CT, DIM], BF16, tag="wrot",
                                         name=f"w{nm}", bufs=2)
                            nc.scalar.dma_start(
                                out=wt, in_=_row_tiles(dram, DIM))
                            wsb[nm] = [wt[:, ct, :] for ct in range(CT)]

                    def emit_a_unit(u):
                        """One Phase-A matmul unit (48 total: 16 kT, 16 v,
                        8 qT, 8 gT)."""
                        if u < 16:
                            t, jh = u // 2, u % 2
                            ps = psA.tile([128, 512], F32, tag="mmA", name="mmA")
                            for ct in range(CT):
                                nc.tensor.matmul(
                                    ps, wsb["k"][ct][:, t * 128:(t + 1) * 128],
                                    s_sb[ct][:, jh * 512:(jh + 1) * 512],
                                    start=(ct == 0), stop=(ct == CT - 1))
                            if u % 2 == 0:
                                nc.scalar.copy(out=kT[t][:, jh * 512:(jh + 1) * 512], in_=ps)
                            else:
                                nc.vector.tensor_copy(out=kT[t][:, jh * 512:(jh + 1) * 512], in_=ps)
                        elif u < 32:
                            t, vh = (u - 16) // 2, (u - 16) % 2
                            ps = psA.tile([128, 512], F32, tag="mmA", name="mmA")
                            for ct in range(CT):
                                nc.tensor.matmul(
                                    ps, s_sb[ct][:, t * 128:(t + 1) * 128],
                                    wsb["v"][ct][:, vh * 512:(vh + 1) * 512],
                                    start=(ct == 0), stop=(ct == CT - 1))
                            if u % 2 == 0:
                                nc.vector.tensor_copy(out=vsb[t][:, vh * 512:(vh + 1) * 512], in_=ps)
                            else:
                                nc.scalar.copy(out=vsb[t][:, vh * 512:(vh + 1) * 512], in_=ps)
                        elif u < 40:
                            t = u - 32
                            ps = psA.tile([128, IBLK], F32, tag="mmA", name="mmA")
                            for ct in range(CT):
                                nc.tensor.matmul(
                                    ps, wsb["q"][ct][:, t * 128:(t + 1) * 128],
                                    si_sb[ct], start=(ct == 0), stop=(ct == CT - 1))
                            nc.scalar.copy(out=qT[t], in_=ps)
                        else:
                            t = u - 40
                            ps = psA.tile([128, IBLK], F32, tag="mmA", name="mmA")
                            for ct in range(CT):
                                nc.tensor.matmul(
                                    ps, wsb["g"][ct][:, t * 128:(t + 1) * 128],
                                    si_sb[ct], start=(ct == 0), stop=(ct == CT - 1))
                            nc.scalar.activation(out=gT[t], in_=ps, func=AF.Sigmoid,
                                                 bias=bgt_sb[:, t:t + 1], scale=1.0)

                    def emit_quarter(qi):
                        """rinv for j-slice qi and the bias correction mul."""
                        QW = 128
                        sl = slice(qi * QW, (qi + 1) * QW)
                        mu = pb.tile([128, QW], F32, tag="mu", name="mu", bufs=3)
                        v4 = pb.tile([128, QW], F32, tag="v4", name="v4", bufs=3)
                        d = pb.tile([128, QW], F32, tag="d", name="d", bufs=3)
                        nc.vector.tensor_scalar_mul(out=mu, in0=stats[:, 0, sl],
                                                    scalar1=1.0 / DPAIR)
                        nc.vector.tensor_scalar_mul(out=v4, in0=stats[:, 1, sl],
                                                    scalar1=1.0 / DPAIR)
                        nc.vector.tensor_mul(out=d, in0=mu, in1=mu)
                        nc.vector.tensor_sub(out=v4, in0=v4, in1=d)  # var
                        nc.scalar.activation(out=v4, in_=v4, func=AF.Sqrt,
                                             bias=eps4[:, 0:1], scale=1.0)
                        rinv = pb.tile([128, QW], BF16, tag="rinv", name="rinv",
                                       bufs=3)
                        with nc.allow_low_precision(reason="rinv scale factor"):
                            nc.vector.reciprocal(out=rinv, in_=v4)
                        rb = _insert_bcast(rinv[:, :], HEADS, 0)  # [128,(16),QW]
                        nc.vector.tensor_mul(out=bias_h[:, :, sl],
                                             in0=bias_h[:, :, sl], in1=rb)

                    CHUNK_PER_Q = 128 // JB
                    a_done = 0
                    for dc in range(NCHUNK):
                        j0 = dc * JB
                        oct = pb.tile([128, JB, 128], BF16, tag="oct",
                                      bufs=OCT_BUFS, name="oct")
                        nc.sync.dma_start(out=oct, in_=pairb[:, j0:j0 + JB, :])
                        octsq = pb.tile([128, JB, 128], BF16, tag="octsq",
                                        bufs=OCT_BUFS, name="octsq")
                        if dc % 2 == 0:
                            nc.vector.tensor_mul(out=octsq, in0=oct, in1=oct)
                        else:
                            nc.scalar.activation(out=octsq, in_=oct,
                                                 func=AF.Square)
                        pproj = psB.tile([128, JB, HEADS + 2], F32, tag="pproj",
                                         bufs=3, name="pproj")
                        for j in range(JB):
                            nc.tensor.matmul(pproj[:, j, 0:HEADS + 1],
                                             oct[:, j, :], weff_sb,
                                             start=True, stop=True)
                            nc.tensor.matmul(pproj[:, j, HEADS + 1:HEADS + 2],
                                             octsq[:, j, :], ones1,
                                             start=True, stop=True)
                        # bias_h[:, h, j0+j] = pproj[:, j, h]
                        bsl = bias_h[:, :, j0:j0 + JB]
                        if dc % 2 == 0:
                            nc.scalar.copy(out=_swap_free(bsl),
                                           in_=pproj[:, :, 0:HEADS])
                        else:
                            nc.vector.tensor_copy(out=_swap_free(bsl),
                                                  in_=pproj[:, :, 0:HEADS])
                        ssl = stats[:, :, j0:j0 + JB]
                        nc.vector.tensor_copy(out=_swap_free(ssl),
                                              in_=pproj[:, :, HEADS:HEADS + 2])
                        if (dc + 1) % CHUNK_PER_Q == 0:
                            emit_quarter(dc // CHUNK_PER_Q)
                        if dc == 3:
                            emit_s_dmas()
                        if dc == 5:
                            emit_weight_dmas([("k", wk), ("v", wv)])
                        if dc == 40:
                            emit_weight_dmas([("q", wq), ("g", wg)])
                        # spread the 48 Phase-A units over chunks 8..NCHUNK
                        if dc >= 8:
                            a_tgt = (dc - 7) * 48 // (NCHUNK - 8)
                            while a_done < a_tgt:
                                emit_a_unit(a_done)
                                a_done += 1

                # ---------------- Phase C: attention ------------------------
                with tc.tile_pool(name="pc", bufs=1) as pc, \
                     tc.tile_pool(name="psC", bufs=2, space="PSUM") as psC:
                    wo4 = pc.tile([128, 8, DIM], BF16, tag="wo4", name="wo4")
                    nc.sync.dma_start(out=wo4, in_=_row_tiles(wo, DIM))
                    wo_sb = [wo4[:, t, :] for t in range(8)]
                    og = [pc.tile([128, IBLK], BF16, tag=f"og{t}", name=f"og{t}") for t in range(8)]
                    zero1 = pc.tile([128, 1], F32, tag="zero1", name="zero1")
                    nc.vector.memset(zero1, 0.0)

                    ot_ps = None
                    for h in range(HEADS):
                        t = h // 2
                        off = 64 * (h % 2)
                        sc_ps = psC.tile([128, N], F32, tag="sc", bufs=2, name="sc")
                        for jh in range(2):
                            sl = slice(jh * 512, (jh + 1) * 512)
                            nc.tensor.matmul(
                                sc_ps[:, sl], qT[t][off:off + 64, :],
                                kT[t][off:off + 64, sl],
                                start=True, stop=False)
                            nc.tensor.matmul(
                                sc_ps[:, sl], ident, bias_h[:, h, sl],
                                start=False, stop=True)
                        # scores are O(10) here: exp() without max-subtraction is
                        # safe in f32, and softmax is shift-invariant.
                        ssum = pc.tile([128, 1], F32, tag="ssum", bufs=3, name="ssum")
                        attn = pc.tile([128, N], BF16, tag="attn", bufs=3, name="attn")
                        nc.scalar.activation(out=attn, in_=sc_ps, func=AF.Exp,
                                             bias=zero1[:, 0:1], scale=1.0,
                                             accum_out=ssum)
                        rs = pc.tile([128, 1], F32, tag="rs", bufs=3, name="rs")
                        nc.vector.reciprocal(out=rs, in_=ssum)
                        nc.vector.tensor_scalar_mul(out=attn, in0=attn, scalar1=rs)
                        if h % 2 == 0:
                            ot_ps = psC.tile([128, IBLK], F32, tag="ot", bufs=2, name="ot")
                        # transpose attn on the (idle-in-C) DMA xbar:
                        # aT8[jj, jt, i] = attn[i, jt*128 + jj]
                        aT8 = pc.tile([128, 8, 128], BF16, tag="aT8", bufs=2,
                                      name="aT8")
                        nc.sync.dma_start(out=aT8, in_=attn[:, :],
                                          transpose=True)
                        for jt in range(8):
                            nc.tensor.matmul(
                                ot_ps[off:off + 64, :],
                                vsb[jt][:, h * 64:(h + 1) * 64], aT8[:, jt, :],
                                start=(jt == 0), stop=(jt == 7))
                        if h % 2 == 1:
                            nc.vector.tensor_mul(out=og[t], in0=ot_ps, in1=gT[t])

                    # out = og^T @ Wo
                    out_sb = pc.tile([128, DIM], F32, tag="out_sb", name="out_sb")
                    for eh in range(2):
                        ps = psC.tile([128, 512], F32, tag="po", bufs=1, name="po")
                        for t in range(8):
                            nc.tensor.matmul(
                                ps, og[t], wo_sb[t][:, eh * 512:(eh + 1) * 512],
                                start=(t == 0), stop=(t == 7))
                        nc.scalar.copy(out=out_sb[:, eh * 512:(eh + 1) * 512], in_=ps)
                    nc.sync.dma_start(out=out[:, :], in_=out_sb[:, 0:out_cols])

    nc.compile()
    return nc


_CACHE = {}


def _prep_inputs(single_repr, pairwise_repr, ln_gamma, ln_beta, W_bias,
                 Wq, Wk, Wv, Wg, bg, Wo):
    sr = np.asarray(single_repr, np.float32).reshape(N, DIM)
    pw = np.asarray(pairwise_repr, np.float32).reshape(N, N, DPAIR)
    pwb = pw.astype(BFNP)
    gamma = np.asarray(ln_gamma, np.float32)
    Wb = np.asarray(W_bias, np.float32)
    weff = gamma[:, None] * Wb                                   # [128, 16]
    # fold the LN mean-correction into the weights:
    #   rinv*(x@weff) - rinv*mu*colsum = rinv * (x @ (weff - colsum/128))
    w2 = weff - weff.sum(0)[None, :] / DPAIR
    weff17 = np.concatenate(
        [w2, np.ones((DPAIR, 1), np.float32)], axis=1)           # [128, 17]
    sT_np = np.ascontiguousarray(sr.T).astype(BFNP)              # [DIM, N]
    scale = DHEAD ** -0.5
    common = {
        "sT": sT_np,
        "wq": (np.asarray(Wq, np.float32) * scale).astype(BFNP),
        "wk": np.asarray(Wk, np.float32).astype(BFNP),
        "wv": np.asarray(Wv, np.float32).astype(BFNP),
        "wg": np.asarray(Wg, np.float32).astype(BFNP),
        "wo": np.asarray(Wo, np.float32).astype(BFNP),
        "weff": weff17.astype(BFNP),
        "bgt": np.ascontiguousarray(
            np.asarray(bg, np.float32).reshape(8, 128).T),
    }
    in_maps = []
    for c in range(NCORES):
        m = dict(common)
        m["pairb"] = np.ascontiguousarray(
            pwb[c * IBLK:(c + 1) * IBLK].transpose(2, 1, 0))
        m["sTi"] = np.ascontiguousarray(sT_np[:, c * IBLK:(c + 1) * IBLK])
        in_maps.append(m)
    return in_maps


def kernel(single_repr, pairwise_repr, ln_gamma, ln_beta, W_bias,
           Wq, Wk, Wv, Wg, bg, Wo, _trace=False):
    if "nc" not in _CACHE:
        _CACHE["nc"] = build_program()
    nc = _CACHE["nc"]
    in_maps = _prep_inputs(single_repr, pairwise_repr, ln_gamma, ln_beta,
                           W_bias, Wq, Wk, Wv, Wg, bg, Wo)
    res = run_bass_kernel_spmd(nc, in_maps, core_ids=list(range(NCORES)),
                               trace=_trace)
    out = np.concatenate([res.results[c]["out"] for c in range(NCORES)], axis=0)
    if _trace:
        kernel.last_result = res
    return out.reshape(1, N, DIM).astype(np.float32)


# revision 26
# speedup vs baseline: 1.0612x; 1.0612x over previous
"""AttentionPairBias kernel for Trainium2, 8-core SPMD.

Math (per batch=1):
  pn        = LayerNorm(pairwise) * gamma + beta                  [N, N, 128]
  attn_bias = einsum('ijp,ph->hij', pn, W_bias)                   [16, N, N]
  q,k,v     = single @ Wq/Wk/Wv  (split into 16 heads of 64)
  scores    = q k^T / sqrt(64) + attn_bias ; attn = softmax_j
  o         = attn @ v ; out = (o * sigmoid(single@Wg + bg)) @ Wo [N, 1024]

Sharding: rows of i (queries) across 8 cores; k/v compute replicated.

v2 design:
  - pairwise is pre-cast to bf16 on the host; the per-core slice
    [128, N, 128] streams in via HWDGE xbar DMA-transpose straight into
    SBUF as oct[p, j, i] tiles (no PE transposes, no PSUM evacuation).
  - LN mean-term is folded into the weights: W2 = gamma*W_bias - colsum/128,
    so bias = rinv * (x @ W2).  The projection matmul streams [W2 | ones]
    (17 cols) per j; a second 1-col matmul vs ones over octsq gives sumsq.
    Both land in one PSUM tile [128, JB, 18]: cols 0-15 bias, 16 sums,
    17 sumsq.  beta@W_bias is constant over j and cancels in softmax.
  - Per-quarter stats produce rinv (bf16); one 2x-mode DVE mul applies it.
  - Phase A (q/k/v/gates) is emitted interleaved with Phase B chunks so its
    matmuls fill the Tensor engine while B is DMA-paced.
  - Phase C softmax: bias is accumulated into the qk PSUM via an
    identity-stationary matmul; ACT exp+accum reads PSUM directly.
"""

import numpy as np
import ml_dtypes

import concourse.bacc as bacc
import concourse.bass as bass
import concourse.tile as tile
import concourse.mybir as mybir
from concourse.bass_utils import run_bass_kernel_spmd
from concourse.masks import make_identity

N, DIM, HEADS, DHEAD, DPAIR = 1024, 1024, 16, 64, 128
NCORES = 8
IBLK = N // NCORES  # 128
EPS = 1e-5

JB = 16             # j-columns per Phase-B chunk
NCHUNK = N // JB
OCT_BUFS = 8

F32 = mybir.dt.float32
BF16 = mybir.dt.bfloat16
AX = mybir.AxisListType
AF = mybir.ActivationFunctionType
OP = mybir.AluOpType
BFNP = ml_dtypes.bfloat16


def _insert_bcast(ap, count, pos):
    """Insert a zero-stride broadcast dim of length `count` at free-dim
    position `pos` (0 = right after the partition dim)."""
    l = list(ap.ap)
    l.insert(1 + pos, [0, count])
    return bass.AP(tensor=ap.tensor, offset=ap.offset, ap=l)


def _row_tiles(dram_t, ncols):
    """AP viewing a [R, ncols] DRAM matrix as [128, R//128, ncols]:
    partition p, free (ct, c) -> dram[ct*128 + p, c]."""
    base = dram_t[:, :]
    nrows = base.ap[0][1]
    return bass.AP(tensor=base.tensor, offset=base.offset,
                   ap=[[ncols, 128], [128 * ncols, nrows // 128], [1, ncols]])


def _swap_free(ap):
    """Swap the two free dims of a 3D AP (iteration-order change)."""
    l = list(ap.ap)
    assert len(l) == 3
    return bass.AP(tensor=ap.tensor, offset=ap.offset, ap=[l[0], l[2], l[1]])


def build_program(reps=1, tiny_out=False):
    nc = bacc.Bacc("TRN2", target_bir_lowering=False, debug=False)

    pairb = nc.dram_tensor("pairb", [DPAIR, N, IBLK], BF16, kind="ExternalInput")
    sT = nc.dram_tensor("sT", [DIM, N], BF16, kind="ExternalInput")
    sTi = nc.dram_tensor("sTi", [DIM, IBLK], BF16, kind="ExternalInput")
    wq = nc.dram_tensor("wq", [DIM, DIM], BF16, kind="ExternalInput")
    wk = nc.dram_tensor("wk", [DIM, DIM], BF16, kind="ExternalInput")
    wv = nc.dram_tensor("wv", [DIM, DIM], BF16, kind="ExternalInput")
    wg = nc.dram_tensor("wg", [DIM, DIM], BF16, kind="ExternalInput")
    wo = nc.dram_tensor("wo", [DIM, DIM], BF16, kind="ExternalInput")
    weff = nc.dram_tensor("weff", [DPAIR, HEADS + 1], BF16, kind="ExternalInput")
    bgt = nc.dram_tensor("bgt", [128, 8], F32, kind="ExternalInput")
    out_cols = 8 if tiny_out else DIM
    out = nc.dram_tensor("out", [IBLK, out_cols], F32, kind="ExternalOutput")

    CT = DIM // 128  # 8 contraction tiles

    with tile.TileContext(nc) as tc:
        with tc.tile_pool(name="consts", bufs=1) as consts, \
             tc.tile_pool(name="persist", bufs=1) as pers:
            ident = consts.tile([128, 128], BF16, tag="ident", name="ident")
            make_identity(nc, ident)
            ones1 = consts.tile([128, 1], BF16, tag="ones1", name="ones1")
            nc.vector.memset(ones1, 1.0)
            weff_sb = consts.tile([DPAIR, HEADS + 1], BF16, tag="weff", name="weff")
            nc.sync.dma_start(out=weff_sb, in_=weff[:, :])
            bgt_sb = consts.tile([128, 8], F32, tag="bgt", name="bgt")
            nc.sync.dma_start(out=bgt_sb, in_=bgt[:, :])

            for _rep in range(reps):
                # persistent tensors
                kT = [pers.tile([128, N], BF16, tag=f"kT{t}", name=f"kT{t}") for t in range(8)]
                vsb = [pers.tile([128, DIM], BF16, tag=f"v{t}", name=f"v{t}") for t in range(8)]
                qT = [pers.tile([128, IBLK], BF16, tag=f"qT{t}", name=f"qT{t}") for t in range(8)]
                gT = [pers.tile([128, IBLK], F32, tag=f"gT{t}", name=f"gT{t}") for t in range(8)]
                bias_h = pers.tile([128, HEADS, N], BF16, tag="biasH", name="biasH")
                stats = pers.tile([128, 2, N], F32, tag="stats", name="stats")

                # ------- Phases B (pairwise bias) + A (projections), fused ----
                with tc.tile_pool(name="pb", bufs=1) as pb, \
                     tc.tile_pool(name="psB", bufs=2, space="PSUM") as psB, \
                     tc.tile_pool(name="pa", bufs=1) as pa, \
                     tc.tile_pool(name="psA", bufs=2, space="PSUM") as psA:
                    eps4 = pb.tile([128, 1], F32, tag="eps4", name="eps4")
                    nc.vector.memset(eps4, EPS)

                    # Phase A inputs (DMAs deferred into the chunk loop so
                    # the pairwise stream gets the DMA engines first)
                    s_sb = []
                    si_sb = []

                    def emit_s_dmas():
                        s4 = pa.tile([128, CT, N], BF16, tag="s4", name="s4")
                        nc.scalar.dma_start(out=s4, in_=_row_tiles(sT, N))
                        s_sb.extend(s4[:, ct, :] for ct in range(CT))
                        si4 = pa.tile([128, CT, IBLK], BF16, tag="si4",
                                      name="si4")
                        nc.scalar.dma_start(out=si4, in_=_row_tiles(sTi, IBLK))
                        si_sb.extend(si4[:, ct, :] for ct in range(CT))
                    wsb = {}

                    def emit_weight_dmas(pairs):
                        # rotating 2-buf tag: k/v fill the two bufs, q/g reuse
                        # them once the kT/v matmul units have drained
                        for nm, dram in pairs:
                            wt = pa.tile([128, CT, DIM], BF16, tag="wrot",
                                         name=f"w{nm}", bufs=2)
                            nc.scalar.dma_start(
                                out=wt, in_=_row_tiles(dram, DIM))
                            wsb[nm] = [wt[:, ct, :] for ct in range(CT)]

                    def emit_a_unit(u):
                        """One Phase-A matmul unit (48 total: 16 kT, 16 v,
                        8 qT, 8 gT)."""
                        if u < 16:
                            t, jh = u // 2, u % 2
                            ps = psA.tile([128, 512], F32, tag="mmA", name="mmA")
                            for ct in range(CT):
                                nc.tensor.matmul(
                                    ps, wsb["k"][ct][:, t * 128:(t + 1) * 128],
                                    s_sb[ct][:, jh * 512:(jh + 1) * 512],
                                    start=(ct == 0), stop=(ct == CT - 1))
                            if u % 2 == 0:
                                nc.scalar.copy(out=kT[t][:, jh * 512:(jh + 1) * 512], in_=ps)
                            else:
                                nc.vector.tensor_copy(out=kT[t][:, jh * 512:(jh + 1) * 512], in_=ps)
                        elif u < 32:
                            t, vh = (u - 16) // 2, (u - 16) % 2
                            ps = psA.tile([128, 512], F32, tag="mmA", name="mmA")
                            for ct in range(CT):
                                nc.tensor.matmul(
                                    ps, s_sb[ct][:, t * 128:(t + 1) * 128],
                                    wsb["v"][ct][:, vh * 512:(vh + 1) * 512],
                                    start=(ct == 0), stop=(ct == CT - 1))
                            if u % 2 == 0:
                                nc.vector.tensor_copy(out=vsb[t][:, vh * 512:(vh + 1) * 512], in_=ps)
                            else:
                                nc.scalar.copy(out=vsb[t][:, vh * 512:(vh + 1) * 512], in_=ps)
                        elif u < 40:
                            t = u - 32
                            ps = psA.tile([128, IBLK], F32, tag="mmA", name="mmA")
                            for ct in range(CT):
                                nc.tensor.matmul(
                                    ps, wsb["q"][ct][:, t * 128:(t + 1) * 128],
                                    si_sb[ct], start=(ct == 0), stop=(ct == CT - 1))
                            nc.scalar.copy(out=qT[t], in_=ps)
                        else:
                            t = u - 40
                            ps = psA.tile([128, IBLK], F32, tag="mmA", name="mmA")
                            for ct in range(CT):
                                nc.tensor.matmul(
                                    ps, wsb["g"][ct][:, t * 128:(t + 1) * 128],
                                    si_sb[ct], start=(ct == 0), stop=(ct == CT - 1))
                            nc.scalar.activation(out=gT[t], in_=ps, func=AF.Sigmoid,
                                                 bias=bgt_sb[:, t:t + 1], scale=1.0)

                    def emit_quarter(qi):
                        """rinv for j-slice qi and the bias correction mul."""
                        QW = 128
                        sl = slice(qi * QW, (qi + 1) * QW)
                        mu = pb.tile([128, QW], F32, tag="mu", name="mu", bufs=3)
                        v4 = pb.tile([128, QW], F32, tag="v4", name="v4", bufs=3)
                        d = pb.tile([128, QW], F32, tag="d", name="d", bufs=3)
                        nc.vector.tensor_scalar_mul(out=mu, in0=stats[:, 0, sl],
                                                    scalar1=1.0 / DPAIR)
                        nc.vector.tensor_scalar_mul(out=v4, in0=stats[:, 1, sl],
                                                    scalar1=1.0 / DPAIR)
                        nc.vector.tensor_mul(out=d, in0=mu, in1=mu)
                        nc.vector.tensor_sub(out=v4, in0=v4, in1=d)  # var
                        nc.scalar.activation(out=v4, in_=v4, func=AF.Sqrt,
                                             bias=eps4[:, 0:1], scale=1.0)
                        rinv = pb.tile([128, QW], BF16, tag="rinv", name="rinv",
                                       bufs=3)
                        with nc.allow_low_precision(reason="rinv scale factor"):
                            nc.vector.reciprocal(out=rinv, in_=v4)
                        rb = _insert_bcast(rinv[:, :], HEADS, 0)  # [128,(16),QW]
                        nc.vector.tensor_mul(out=bias_h[:, :, sl],
                                             in0=bias_h[:, :, sl], in1=rb)

                    CHUNK_PER_Q = 128 // JB
                    a_done = 0
                    for dc in range(NCHUNK):
                        j0 = dc * JB
                        oct = pb.tile([128, JB, 128], BF16, tag="oct",
                                      bufs=OCT_BUFS, name="oct")
                        nc.sync.dma_start(out=oct, in_=pairb[:, j0:j0 + JB, :])
                        octsq = pb.tile([128, JB, 128], BF16, tag="octsq",
                                        bufs=OCT_BUFS, name="octsq")
                        if dc % 2 == 0:
                            nc.vector.tensor_mul(out=octsq, in0=oct, in1=oct)
                        else:
                            nc.scalar.activation(out=octsq, in_=oct,
                                                 func=AF.Square)
                        pproj = psB.tile([128, JB, HEADS + 2], F32, tag="pproj",
                                         bufs=3, name="pproj")
                        for j in range(JB):
                            nc.tensor.matmul(pproj[:, j, 0:HEADS + 1],
                                             oct[:, j, :], weff_sb,
                                             start=True, stop=True)
                            nc.tensor.matmul(pproj[:, j, HEADS + 1:HEADS + 2],
                                             octsq[:, j, :], ones1,
                                             start=True, stop=True)
                        # bias_h[:, h, j0+j] = pproj[:, j, h]
                        bsl = bias_h[:, :, j0:j0 + JB]
                        if dc % 2 == 0:
                            nc.scalar.copy(out=_swap_free(bsl),
                                           in_=pproj[:, :, 0:HEADS])
                        else:
                            nc.vector.tensor_copy(out=_swap_free(bsl),
                                                  in_=pproj[:, :, 0:HEADS])
                        ssl = stats[:, :, j0:j0 + JB]
                        nc.vector.tensor_copy(out=_swap_free(ssl),
                                              in_=pproj[:, :, HEADS:HEADS + 2])
                        if (dc + 1) % CHUNK_PER_Q == 0:
                            emit_quarter(dc // CHUNK_PER_Q)
                        if dc == 3:
                            emit_s_dmas()
                        if dc == 5:
                            emit_weight_dmas([("k", wk), ("v", wv)])
                        if dc == 40:
                            emit_weight_dmas([("q", wq), ("g", wg)])
                        # spread the 48 Phase-A units over chunks 8..NCHUNK
                        if dc >= 8:
                            a_tgt = (dc - 7) * 48 // (NCHUNK - 8)
                            while a_done < a_tgt:
                                emit_a_unit(a_done)
                                a_done += 1

                # ---------------- Phase C: attention ------------------------
                with tc.tile_pool(name="pc", bufs=1) as pc, \
                     tc.tile_pool(name="psC", bufs=2, space="PSUM") as psC:
                    wo4 = pc.tile([128, 8, DIM], BF16, tag="wo4", name="wo4")
                    nc.sync.dma_start(out=wo4, in_=_row_tiles(wo, DIM))
                    wo_sb = [wo4[:, t, :] for t in range(8)]
                    og = [pc.tile([128, IBLK], BF16, tag=f"og{t}", name=f"og{t}") for t in range(8)]
                    zero1 = pc.tile([128, 1], F32, tag="zero1", name="zero1")
                    nc.vector.memset(zero1, 0.0)

                    ot_ps = None
                    for h in range(HEADS):
                        t = h // 2
                        off = 64 * (h % 2)
                        sc_ps = psC.tile([128, N], F32, tag="sc", bufs=2, name="sc")
                        for jh in range(2):
                            sl = slice(jh * 512, (jh + 1) * 512)
                            nc.tensor.matmul(
                                sc_ps[:, sl], qT[t][off:off + 64, :],
                                kT[t][off:off + 64, sl],
                                start=True, stop=False)
                            nc.tensor.matmul(
                                sc_ps[:, sl], ident, bias_h[:, h, sl],
                                start=False, stop=True)
                        # scores are O(10) here: exp() without max-subtraction is
                        # safe in f32, and softmax is shift-invariant.
                        ssum = pc.tile([128, 1], F32, tag="ssum", bufs=3, name="ssum")
                        attn = pc.tile([128, N], BF16, tag="attn", bufs=3, name="attn")
                        nc.scalar.activation(out=attn, in_=sc_ps, func=AF.Exp,
                                             bias=zero1[:, 0:1], scale=1.0,
                                             accum_out=ssum)
                        rs = pc.tile([128, 1], F32, tag="rs", bufs=3, name="rs")
                        nc.vector.reciprocal(out=rs, in_=ssum)
                        nc.vector.tensor_scalar_mul(out=attn, in0=attn, scalar1=rs)
                        if h % 2 == 0:
                            ot_ps = psC.tile([128, IBLK], F32, tag="ot", bufs=1, name="ot")
                        for j4 in range(2):
                            pT = psC.tile([128, 4, 128], BF16, tag="pT", bufs=2, name="pT")
                            for jj in range(4):
                                jt = j4 * 4 + jj
                                nc.tensor.transpose(
                                    pT[:, jj, :],
                                    attn[:, jt * 128:(jt + 1) * 128], ident)
                            aT = pc.tile([128, 4, 128], BF16, tag="aT", bufs=4, name="aT")
                            if j4 % 2 == 0:
                                nc.vector.tensor_copy(out=aT.bitcast(F32),
                                                      in_=pT.bitcast(F32))
                            else:
                                nc.scalar.copy(out=aT.bitcast(F32),
                                               in_=pT.bitcast(F32))
                            for jj in range(4):
                                jt = j4 * 4 + jj
                                nc.tensor.matmul(
                                    ot_ps[off:off + 64, :],
                                    vsb[jt][:, h * 64:(h + 1) * 64], aT[:, jj, :],
                                    start=(jt == 0), stop=(jt == 7))
                        if h % 2 == 1:
                            nc.vector.tensor_mul(out=og[t], in0=ot_ps, in1=gT[t])

                    # out = og^T @ Wo
                    out_sb = pc.tile([128, DIM], F32, tag="out_sb", name="out_sb")
                    for eh in range(2):
                        ps = psC.tile([128, 512], F32, tag="po", bufs=1, name="po")
                        for t in range(8):
                            nc.tensor.matmul(
                                ps, og[t], wo_sb[t][:, eh * 512:(eh + 1) * 512],
                                start=(t == 0), stop=(t == 7))
                        nc.scalar.copy(out=out_sb[:, eh * 512:(eh + 1) * 512], in_=ps)
                    nc.sync.dma_start(out=out[:, :], in_=out_sb[:, 0:out_cols])

    nc.compile()
    return nc


_CACHE = {}


def _prep_inputs(single_repr, pairwise_repr, ln_gamma, ln_beta, W_bias,
                 Wq, Wk, Wv, Wg, bg, Wo):
    sr = np.asarray(single_repr, np.float32).reshape(N, DIM)
    pw = np.asarray(pairwise_repr, np.float32).reshape(N, N, DPAIR)
    pwb = pw.astype(BFNP)
    gamma = np.asarray(ln_gamma, np.float32)
    Wb = np.asarray(W_bias, np.float32)
    weff = gamma[:, None] * Wb                                   # [128, 16]
    # fold the LN mean-correction into the weights:
    #   rinv*(x@weff) - rinv*mu*colsum = rinv * (x @ (weff - colsum/128))
    w2 = weff - weff.sum(0)[None, :] / DPAIR
    weff17 = np.concatenate(
        [w2, np.ones((DPAIR, 1), np.float32)], axis=1)           # [128, 17]
    sT_np = np.ascontiguousarray(sr.T).astype(BFNP)              # [DIM, N]
    scale = DHEAD ** -0.5
    common = {
        "sT": sT_np,
        "wq": (np.asarray(Wq, np.float32) * scale).astype(BFNP),
        "wk": np.asarray(Wk, np.float32).astype(BFNP),
        "wv": np.asarray(Wv, np.float32).astype(BFNP),
        "wg": np.asarray(Wg, np.float32).astype(BFNP),
        "wo": np.asarray(Wo, np.float32).astype(BFNP),
        "weff": weff17.astype(BFNP),
        "bgt": np.ascontiguousarray(
            np.asarray(bg, np.float32).reshape(8, 128).T),
    }
    in_maps = []
    for c in range(NCORES):
        m = dict(common)
        m["pairb"] = np.ascontiguousarray(
            pwb[c * IBLK:(c + 1) * IBLK].transpose(2, 1, 0))
        m["sTi"] = np.ascontiguousarray(sT_np[:, c * IBLK:(c + 1) * IBLK])
        in_maps.append(m)
    return in_maps


def kernel(single_repr, pairwise_repr, ln_gamma, ln_beta, W_bias,
           Wq, Wk, Wv, Wg, bg, Wo, _trace=False):
    if "nc" not in _CACHE:
        _CACHE["nc"] = build_program()
    nc = _CACHE["nc"]
    in_maps = _prep_inputs(single_repr, pairwise_repr, ln_gamma, ln_beta,
                           W_bias, Wq, Wk, Wv, Wg, bg, Wo)
    res = run_bass_kernel_spmd(nc, in_maps, core_ids=list(range(NCORES)),
                               trace=_trace)
    out = np.concatenate([res.results[c]["out"] for c in range(NCORES)], axis=0)
    if _trace:
        kernel.last_result = res
    return out.reshape(1, N, DIM).astype(np.float32)
